# revision 19
# baseline (speedup 1.0000x reference)
"""Causal multi-head self-attention (B=2, S=4096, D=1024, H=16, dk=64) on 8 trn2 cores.

Sharding: core c handles batch b = c // 4 and heads [4*(c%4) .. 4*(c%4)+3]
(data parallel on B, tensor parallel on heads / QKV / O projections).
Each core returns a partial [S, D] output (its heads' contribution after the
Wo projection); the host sums the 4 partials per batch.

Device-side layout choices (per core):
  - host supplies xT = x[b].T so every projection contracts over the model dim
    on partitions without any on-device transpose of x.
  - Wq/Wk columns are permuted on the host so RoPE becomes rotate-half form
    with all-evens on one 128-partition chunk and all-odds on the other
    (full-width DVE ops), then an SBUF->SBUF DMA re-groups the rotated rows
    head-contiguously for the K=64 QK^T contraction.
  - attention runs in S^T layout (scores [k, q]): softmax denominator comes
    from a ones-column appended to V (PV matmul row 64 = sum_k exp), so no
    partition-axis reduction is ever needed; each (head, q-tile) tile of O^T
    is normalized right after its PV accumulation and spilled to a DRAM
    scratch, read back for the final Wo projection.
  - all matmuls use float32r (full PE rate at N>=256, ~fp32 precision).
  - causal mask: one fused tensor_mask DVE op per diagonal 128x512 block,
    applied to exp(S^T) (masked lanes -> exact 0).
"""

import numpy as np
from ml_dtypes import bfloat16

import concourse.bass as bass
import concourse.bacc as bacc
import concourse.mybir as mybir
import concourse.tile as tile
from concourse.bass_utils import run_bass_kernel_spmd

P = 128
D_MODEL = 1024
N_HEADS = 16
D_K = 64
SEQ = 4096
BATCH = 2
N_CORES = 8
HEADS_PER_CORE = 4
ST = 512  # s-tile / q-tile width
THETA = 10000.0

f32 = mybir.dt.float32
f32r = mybir.dt.float32r
bf16 = mybir.dt.bfloat16
AF = mybir.ActivationFunctionType
OP = mybir.AluOpType


def build_program(S=SEQ, debug_taps=False, reps=1):
    """Build the single-core SPMD Bass program (same program on all cores).

    Fully fused pipeline, one pass over 512-wide s-tiles: project Q/K/V for
    tile t (Q transient, K/V appended to the resident KV cache), run causal
    attention for q-tile t over all 4 heads (keys 0..t ready), normalize O^T
    into an SBUF accumulator, and apply the Wo projection for those 4 output
    row-blocks immediately. All PSUM flows through one 3-bank rotating tag
    (projections, score staging, Wo) plus a 1-bank O^T accumulator, so exp
    batches 3 key-blocks per ACT instruction.
    """
    nc = bacc.Bacc("TRN2", target_bir_lowering=False, debug=False,
                   num_devices=N_CORES)

    NT = S // ST          # number of s-tiles == number of q-tiles
    NKB = S // P          # number of 128-wide key blocks
    NIC = D_MODEL // P    # contraction chunks over the model dim

    xT_d = nc.dram_tensor("xT", [D_MODEL, S], bf16, kind="ExternalInput").ap()
    wqkv_d = nc.dram_tensor("wqkvT", [D_MODEL, 768], bf16, kind="ExternalInput").ap()
    wo_d = nc.dram_tensor("woT", [256, D_MODEL], bf16, kind="ExternalInput").ap()
    cos_d = nc.dram_tensor("cos4", [P, S], bf16, kind="ExternalInput").ap()
    sin_d = nc.dram_tensor("sin4", [P, S], bf16, kind="ExternalInput").ap()
    mask_d = nc.dram_tensor("dmask", [P, P], bf16, kind="ExternalInput").ap()
    y_d = nc.dram_tensor("y", [S, D_MODEL], f32, kind="ExternalOutput").ap()
    GK = 2  # key-blocks per exp/staging group

    with tile.TileContext(nc) as tc:
      for _rep in range(reps):
        with tc.tile_pool(name="res", bufs=1) as res:
            KT = res.tile([P, 2, S], bf16)     # rotated K^T, head-contiguous
            V = res.tile([P, NKB, 260], bf16)  # V blocks, 65 cols/head (ones col)

            v4 = V.rearrange("p k (h c) -> p k h c", h=HEADS_PER_CORE)
            # memset can't write f32r; set the ones column via a u32 bitcast
            nc.vector.memset(v4[:, :, :, 64:65].bitcast(mybir.dt.uint16),
                             0x3F80)

            with tc.tile_pool(name="p1", bufs=2) as p1, \
                 tc.tile_pool(name="rot", bufs=2) as rotp, \
                 tc.tile_pool(name="p2", bufs=2) as p2, \
                 tc.tile_pool(name="ppp", bufs=2, space="PSUM") as ppp, \
                 tc.tile_pool(name="stg", bufs=2, space="PSUM") as stgp, \
                 tc.tile_pool(name="opp", bufs=2, space="PSUM") as opp:
                xT3 = xT_d.rearrange("(a p) s -> p a s", p=P)
                wq3 = wqkv_d.rearrange("(a p) f -> p a f", p=P)
                # prologue: interleave the first x tile's per-ic chunks with
                # the wqkv chunks so the first projection matmuls can start
                # ~2us in instead of after the full weight load; wo/mask are
                # needed later, so they load last
                wqkv_sb = p1.tile([P, NIC, 768], bf16, tag="wqkv", bufs=1)
                xt0 = p1.tile([P, NIC, ST], bf16, tag="xt", bufs=2)
                for _ic in range(NIC):
                    nc.sync.dma_start(xt0[:, _ic, :], xT3[:, _ic, 0:ST])
                    nc.sync.dma_start(wqkv_sb[:, _ic, :], wq3[:, _ic, :])
                cs0 = p1.tile([P, 2, ST], bf16, tag="cs")
                nc.sync.dma_start(cs0[:, 0, :], cos_d[:, 0:ST])
                nc.sync.dma_start(cs0[:, 1, :], sin_d[:, 0:ST])
                mask_sb = p2.tile([P, P], bf16, tag="mask", bufs=1)
                nc.sync.dma_start(mask_sb, mask_d)

                def emit_wo_sb(qt_prev, ot_prev, sb):
                    # one 128-row block of the Wo projection for q-tile qt_prev
                    out_t = p2.tile([P, D_MODEL], f32, tag="y", bufs=2)
                    for nh in (0, 1):
                        y_ps = ppp.tile([P, 512], f32, tag="pp")
                        for j in (0, 1):
                            nc.tensor.matmul(
                                y_ps,
                                lhsT=ot_prev[:, j, sb * P:(sb + 1) * P],
                                rhs=wo_sb[:, j, nh * 512:(nh + 1) * 512],
                                start=(j == 0), stop=(j == 1))
                        nc.vector.tensor_copy(
                            out_t[:, nh * 512:(nh + 1) * 512], y_ps)
                        nc.sync.dma_start(
                            y_d[qt_prev * ST + sb * P:
                                qt_prev * ST + (sb + 1) * P,
                                nh * 512:(nh + 1) * 512],
                            out_t[:, nh * 512:(nh + 1) * 512])

                def bg_gen(tt, xt, cs, qt_tile, wo_qt, wo_ot):
                    """Generator for the background work running under the
                    attention of tile tt-1: Q/K/V projections + RoPE for tile
                    tt (if any), then the Wo projection of tile tt-2.  Yields
                    between small PE units so the caller can interleave them
                    with attention score/PV groups (PE executes in program
                    order; this keeps every engine fed)."""
                    if tt is not None:
                        tsl = slice(tt * ST, (tt + 1) * ST)
                        for dst, col0, dsl in ((qt_tile, 0, slice(0, ST)),
                                               (KT, 256, tsl)):
                            pe_ps = ppp.tile([P, ST], f32, tag="pp")
                            po_ps = ppp.tile([P, ST], f32, tag="pp")
                            for ic in range(NIC):
                                nc.tensor.matmul(
                                    pe_ps,
                                    lhsT=wqkv_sb[:, ic, col0:col0 + P],
                                    rhs=xt[:, ic, :],
                                    start=(ic == 0), stop=(ic == NIC - 1))
                                nc.tensor.matmul(
                                    po_ps,
                                    lhsT=wqkv_sb[:, ic, col0 + P:col0 + 256],
                                    rhs=xt[:, ic, :],
                                    start=(ic == 0), stop=(ic == NIC - 1))
                                yield
                            t1 = rotp.tile([P, ST], f32, tag="tmp", bufs=5)
                            t3 = rotp.tile([P, ST], f32, tag="tmp", bufs=5)
                            nc.vector.tensor_tensor(t1, pe_ps, cs[:, 0, :], OP.mult)
                            nc.vector.tensor_tensor(t3, pe_ps, cs[:, 1, :], OP.mult)
                            t2 = rotp.tile([P, ST], f32, tag="tmp", bufs=5)
                            t4 = rotp.tile([P, ST], f32, tag="tmp", bufs=5)
                            nc.vector.tensor_tensor(t2, po_ps, cs[:, 1, :], OP.mult)
                            nc.vector.tensor_tensor(t4, po_ps, cs[:, 0, :], OP.mult)
                            rot_e = rotp.tile([P, ST], bf16, tag="re")
                            rot_o = rotp.tile([P, ST], bf16, tag="ro")
                            nc.vector.tensor_tensor(rot_e, t1, t2, OP.subtract)
                            nc.vector.tensor_tensor(rot_o, t3, t4, OP.add)
                            # scatter rotated rows head-contiguously:
                            # head h evens -> dst[(h%2)*64 +  0 .. +32, h//2, dsl]
                            # head h odds  -> dst[(h%2)*64 + 32 .. +64, h//2, dsl]
                            for j in (0, 1):
                                for hh in (0, 1):
                                    h = 2 * j + hh
                                    nc.sync.dma_start(
                                        dst[hh * 64:hh * 64 + 32, j, dsl],
                                        rot_e[h * 32:(h + 1) * 32, :])
                                    nc.sync.dma_start(
                                        dst[hh * 64 + 32:hh * 64 + 64, j, dsl],
                                        rot_o[h * 32:(h + 1) * 32, :])
                            yield
                        for sb in range(ST // P):
                            kb = tt * (ST // P) + sb
                            v_ps = ppp.tile([P, 256], f32, tag="pp")
                            for ic in range(NIC):
                                nc.tensor.matmul(
                                    v_ps,
                                    lhsT=xt[:, ic, sb * P:(sb + 1) * P],
                                    rhs=wqkv_sb[:, ic, 512:768],
                                    start=(ic == 0), stop=(ic == NIC - 1))
                            nc.vector.tensor_copy(
                                v4[:, kb, :, 0:64],
                                v_ps.rearrange("p (h c) -> p h c",
                                               h=HEADS_PER_CORE))
                            yield
                    if wo_ot is not None:
                        for sb in range(ST // P):
                            emit_wo_sb(wo_qt, wo_ot, sb)
                            yield

                def attention(qt, cur_qt, bg, n_bg_units):
                    nkb = (qt + 1) * (ST // P)
                    n_slots = HEADS_PER_CORE * ((nkb + GK - 1) // GK)
                    slot = 0
                    consumed = 0
                    ot_acc = p2.tile([P, 2, ST], bf16, tag="ota", bufs=2)
                    for h in range(HEADS_PER_CORE):
                        j, hb = h // 2, (h % 2) * 64
                        o_ps = opp.tile([65, ST], f32, tag="o")
                        for g0 in range(0, nkb, GK):
                            glen = min(GK, nkb - g0)
                            # blocks entirely in the upper-right quarter of the
                            # diagonal band only need query columns [256:512)
                            q0 = 256 if g0 - qt * (ST // P) >= 2 else 0
                            stg = stgp.tile([P, GK, ST], f32, tag="s")
                            for gi in range(glen):
                                kb = g0 + gi
                                nc.tensor.matmul(
                                    stg[:, gi, q0:],
                                    lhsT=KT[hb:hb + 64, j, kb * P:(kb + 1) * P],
                                    rhs=cur_qt[hb:hb + 64, j, q0:],
                                    start=True, stop=True)
                            es = p2.tile([P, GK, ST], bf16, tag="e", bufs=4)
                            nc.scalar.activation(es[:, :glen, q0:],
                                                 stg[:, :glen, q0:], AF.Exp)
                            # spread background units evenly over the slots,
                            # emitted between exp and PV: PE is in-order, so
                            # filler placed here hides the exp latency that PV
                            # waits on (after PV it would never run early)
                            slot += 1
                            want = n_bg_units * slot // n_slots
                            while consumed < want:
                                next(bg, None)
                                consumed += 1
                            for gi in range(glen):
                                kb = g0 + gi
                                c = kb - qt * (ST // P)
                                if c >= 0:
                                    # the masked region of diagonal block c is
                                    # the [i, jq] triangle jq < i + 128*c; its
                                    # ragged part lies in cols [128c, 128c+128)
                                    # (one shared 128x128 tri mask); cols < 128c
                                    # are excluded from the PV stream instead
                                    eng = nc.vector if c < 2 else nc.gpsimd
                                    eng.tensor_tensor(
                                        es[:, gi, c * P:(c + 1) * P],
                                        es[:, gi, c * P:(c + 1) * P],
                                        mask_sb, OP.mult)
                                qlo = c * P if c > 0 else 0
                                nc.tensor.matmul(
                                    o_ps[:, qlo:],
                                    lhsT=V[:, kb, h * 65:(h + 1) * 65],
                                    rhs=es[:, gi, qlo:],
                                    start=(kb == 0), stop=(kb == nkb - 1))
                        # normalize this O^T tile into the SBUF accumulator
                        # (reciprocal_approx_fast misbehaves on a PSUM source
                        # at partition base 64 — stage the l row in SBUF first;
                        # the partition broadcast runs on Pool)
                        lr_t = p2.tile([1, ST], f32, tag="lr", bufs=2)
                        nc.vector.tensor_copy(lr_t, o_ps[64:65, :])
                        rt = p2.tile([1, ST], f32, tag="rt", bufs=2)
                        nc.vector.reciprocal_approx_fast(rt, lr_t)
                        rl = p2.tile([64, ST], f32, tag="rl", bufs=2)
                        nc.gpsimd.partition_broadcast(rl, rt)
                        nc.vector.tensor_tensor(ot_acc[hb:hb + 64, j, :],
                                                o_ps[0:64, :], rl, OP.mult)
                    for _ in bg:  # drain leftover background work
                        pass
                    return ot_acc

                # peeled prologue: projections for tile 0 (nothing to overlap
                # with yet), plus the tile-1 input prefetch
                qt0 = p2.tile([P, 2, ST], bf16, tag="qt", bufs=2)
                if NT > 1:
                    xt1 = p1.tile([P, NIC, ST], bf16, tag="xt", bufs=2)
                    nc.sync.dma_start(xt1, xT3[:, :, ST:2 * ST])
                    cs1 = p1.tile([P, 2, ST], bf16, tag="cs")
                    nc.sync.dma_start(cs1[:, 0, :], cos_d[:, ST:2 * ST])
                    nc.sync.dma_start(cs1[:, 1, :], sin_d[:, ST:2 * ST])
                    pending = {1: (xt1, cs1)}
                wo_sb = p1.tile([P, 2, D_MODEL], bf16, tag="wo", bufs=1)
                nc.sync.dma_start(wo_sb, wo_d.rearrange("(a p) f -> p a f", p=P))
                for _ in bg_gen(0, xt0, cs0, qt0, None, None):
                    pass

                prev_ot = None
                cur_qt = qt0
                for t in range(NT):
                    if t + 2 < NT:
                        fsl = slice((t + 2) * ST, (t + 3) * ST)
                        xt_f = p1.tile([P, NIC, ST], bf16, tag="xt", bufs=2)
                        nc.sync.dma_start(xt_f, xT3[:, :, fsl])
                        cs_f = p1.tile([P, 2, ST], bf16, tag="cs")
                        nc.sync.dma_start(cs_f[:, 0, :], cos_d[:, fsl])
                        nc.sync.dma_start(cs_f[:, 1, :], sin_d[:, fsl])
                        pending[t + 2] = (xt_f, cs_f)
                    if t + 1 < NT:
                        xt_n, cs_n = pending.pop(t + 1)
                        next_qt = p2.tile([P, 2, ST], bf16, tag="qt", bufs=2)
                        bg = bg_gen(t + 1, xt_n, cs_n, next_qt,
                                    t - 1, prev_ot)
                    else:
                        next_qt = None
                        bg = bg_gen(None, None, None, None, t - 1, prev_ot)
                    n_units = (22 if t + 1 < NT else 0) + (4 if t >= 1 else 0)
                    prev_ot = attention(t, cur_qt, bg, n_units)
                    cur_qt = next_qt

                for sb in range(ST // P):
                    emit_wo_sb(NT - 1, prev_ot, sb)


    nc.compile()
    return nc


def _round_fp32r(a):
    """Round fp32 to the fp32r format (1s + 8e + 11m in the top 20 bits, RNE).

    The PE consumes float32r operands pre-rounded to 11 mantissa bits; doing
    the rounding on the host makes DMA-fed operands valid fp32r producers.
    """
    b = np.ascontiguousarray(a, dtype=np.float32).view(np.uint32)
    lsb = (b >> np.uint32(12)) & np.uint32(1)
    r = (b + np.uint32(0x7FF) + lsb) & np.uint32(0xFFFFF000)
    return r.view(np.float32)


def make_core_inputs(x, token_positions, Wq, Wk, Wv, Wo, S=SEQ):
    """Host-side sharding/layout prep. Returns in_maps for the 8 cores."""
    x = np.asarray(x, dtype=np.float32)
    Wq = np.asarray(Wq, dtype=np.float32)
    Wk = np.asarray(Wk, dtype=np.float32)
    Wv = np.asarray(Wv, dtype=np.float32)
    Wo = np.asarray(Wo, dtype=np.float32)
    pos = np.asarray(token_positions).astype(np.float32)

    scale = np.float32(1.0 / np.sqrt(np.float32(D_K)))
    half = D_K // 2
    inv_freq = (1.0 / (np.float32(THETA) **
                       (np.arange(0, D_K, 2, dtype=np.float32) / np.float32(D_K))
                       )).astype(np.float32)
    freqs = pos[:, None] * inv_freq[None, :]          # [S, 32]
    cosT = np.cos(freqs).T.astype(np.float32)         # [32, S]
    sinT = np.sin(freqs).T.astype(np.float32)
    cos4 = np.ascontiguousarray(np.tile(cosT, (HEADS_PER_CORE, 1)))  # [128, S]
    sin4 = np.ascontiguousarray(np.tile(sinT, (HEADS_PER_CORE, 1)))

    # shared 128x128 triangular mask for every diagonal block: tri[i, j] = j >= i
    ii = np.arange(P)[:, None]
    jj = np.arange(P)[None, :]
    dmask = np.ascontiguousarray((jj >= ii).astype(np.float32))

    xTs = [np.ascontiguousarray(x[b].T) for b in range(BATCH)]      # [D, S]

    in_maps = []
    for c in range(N_CORES):
        b, g = c // 4, c % 4
        # permutation: wq/wk output dims -> [all 4 heads' evens | all odds]
        perm = np.empty(256, dtype=np.int64)
        for t in range(HEADS_PER_CORE):
            hg = HEADS_PER_CORE * g + t
            perm[t * half:(t + 1) * half] = hg * D_K + 2 * np.arange(half)
            perm[128 + t * half:128 + (t + 1) * half] = \
                hg * D_K + 2 * np.arange(half) + 1
        wqT = (Wq[perm, :] * scale).T                               # [D, 256]
        wkT = Wk[perm, :].T                                         # [D, 256]
        wvT = Wv[g * 256:(g + 1) * 256, :].T                        # [D, 256]
        wqkvT = np.ascontiguousarray(
            np.concatenate([wqT, wkT, wvT], axis=1).astype(np.float32))
        woT = np.ascontiguousarray(Wo[:, g * 256:(g + 1) * 256].T.astype(np.float32))
        in_maps.append({
            "xT": xTs[b].astype(bfloat16),
            "wqkvT": wqkvT.astype(bfloat16),
            "woT": woT.astype(bfloat16),
            "cos4": cos4.astype(bfloat16),
            "sin4": sin4.astype(bfloat16),
            "dmask": dmask.astype(bfloat16),
        })
    return in_maps


_PROGRAM_CACHE = {}


def _get_program(S=SEQ):
    if S not in _PROGRAM_CACHE:
        _PROGRAM_CACHE[S] = build_program(S)
    return _PROGRAM_CACHE[S]


def run_cores(in_maps, trace=False, **kwargs):
    nc = _get_program(SEQ)
    return run_bass_kernel_spmd(nc, in_maps, core_ids=list(range(N_CORES)),
                                trace=trace, **kwargs)


def kernel(x, token_positions, Wq, Wk, Wv, Wo):
    in_maps = make_core_inputs(x, token_positions, Wq, Wk, Wv, Wo)
    res = run_cores(in_maps)
    out = np.zeros((BATCH, SEQ, D_MODEL), dtype=np.float32)
    for c in range(N_CORES):
        out[c // 4] += res.results[c]["y"]
    return out


_TIMED_CACHE = {}


def run_cores_timed(in_maps, iters=8, program=None):
    """Execute the SPMD program with device-resident inputs repeatedly and
    return (per-exec wall seconds list, outputs-per-core). Used for timing
    only — the NTFF profiling hook is unavailable under this axon client."""
    import time

    import jax
    from jax.experimental.shard_map import shard_map
    from jax.sharding import Mesh, NamedSharding, PartitionSpec

    from concourse.bass2jax import (
        _bass_exec_p,
        install_neuronx_cc_hook,
        partition_id_tensor,
    )

    nc = program if program is not None else _get_program(SEQ)

    if id(nc) in _TIMED_CACHE:
        sharded, dev_in, out_avals, out_names, n_cores = _TIMED_CACHE[id(nc)]
        out = sharded(*dev_in)
        jax.block_until_ready(out)
        times = []
        for _ in range(iters):
            t0 = time.perf_counter()
            out = sharded(*dev_in)
            jax.block_until_ready(out)
            times.append(time.perf_counter() - t0)
        results = [
            {name: np.asarray(out[i]).reshape(n_cores, *out_avals[i].shape)[c]
             for i, name in enumerate(out_names)}
            for c in range(n_cores)
        ]
        return times, results
    install_neuronx_cc_hook()
    partition_name = nc.partition_id_tensor.name if nc.partition_id_tensor else None
    in_names, out_names, out_avals, zero_outs = [], [], [], []
    for alloc in nc.m.functions[0].allocations:
        if not isinstance(alloc, mybir.MemoryLocationSet):
            continue
        name = alloc.memorylocations[0].name
        if alloc.kind == "ExternalInput":
            if name != partition_name:
                in_names.append(name)
        elif alloc.kind == "ExternalOutput":
            out_names.append(name)
            shape = tuple(alloc.tensor_shape)
            dtype = mybir.dt.np(alloc.dtype)
            out_avals.append(jax.core.ShapedArray(shape, dtype))
            zero_outs.append(np.zeros(shape, dtype))
    n_params = len(in_names)
    all_names = in_names + out_names + ([partition_name] if partition_name else [])

    def _body(*args):
        operands = list(args)
        if partition_name:
            operands.append(partition_id_tensor())
        outs = _bass_exec_p.bind(
            *operands,
            out_avals=tuple(out_avals),
            in_names=tuple(all_names),
            out_names=tuple(out_names),
            lowering_input_output_aliases=(),
            sim_require_finite=True,
            sim_require_nnan=True,
            nc=nc,
        )
        return tuple(outs)

    n_cores = len(in_maps)
    devices = jax.devices()[:n_cores]
    mesh = Mesh(np.asarray(devices), ("core",))
    nin = n_params + len(out_names)
    sharded = jax.jit(
        shard_map(_body, mesh=mesh,
                  in_specs=(PartitionSpec("core"),) * nin,
                  out_specs=(PartitionSpec("core"),) * len(out_names),
                  check_rep=False),
        keep_unused=True)
    per_core = [[np.asarray(m[n]) for n in in_names] for m in in_maps]
    concat_in = [np.concatenate([per_core[c][i] for c in range(n_cores)], axis=0)
                 for i in range(n_params)]
    concat_zeros = [np.zeros((n_cores * z.shape[0], *z.shape[1:]), z.dtype)
                    for z in zero_outs]
    sh = NamedSharding(mesh, PartitionSpec("core"))
    dev_in = [jax.device_put(a, sh) for a in concat_in + concat_zeros]
    _TIMED_CACHE[id(nc)] = (sharded, dev_in, out_avals, out_names, n_cores)
    out = sharded(*dev_in)
    jax.block_until_ready(out)
    times = []
    for _ in range(iters):
        t0 = time.perf_counter()
        out = sharded(*dev_in)
        jax.block_until_ready(out)
        times.append(time.perf_counter() - t0)
    results = [
        {name: np.asarray(out[i]).reshape(n_cores, *out_avals[i].shape)[c]
         for i, name in enumerate(out_names)}
        for c in range(n_cores)
    ]
    return times, results

